# revision 56
# baseline (speedup 1.0000x reference)
# Trainium2 Bass kernel for nn_CapLayer (CapsNet grouped 1x1 conv + dynamic routing).
#
# Key algebraic restructuring: the huge intermediate pred[b, i=(g,s), (j,d)]
# (188MB for the full batch) is NEVER materialized. Routing is computed in a
# factored form:
#   pred[b,(g,s),(j,d)] = sum_c Wa[g,j,d,c] * xga[b,g,c,s]     (c augmented with
#                                                               a ones channel to
#                                                               absorb the bias)
#   t[b,j,g,c]  = sum_s c[b,j,(g,s)] * xga[b,g,c,s]
#   s[b,j,d]    = sum_{g,c} t[b,j,g,c] * Wa[g,j,d,c]
#   u[b,j,g,c]  = sum_d v[b,j,d] * Wa[g,j,d,c]
#   db[b,j,g,s] = sum_c u[b,j,g,c] * xga[b,g,c,s]
# Iteration 1 collapses (softmax of zeros is uniform): t1 = sum_s x / J,
# which ships in precomputed with the host-side input marshalling.
# The squash factor f = |s|/(1+|s|^2) is folded into u (u = f * (s @ W))
# instead of materializing v, which takes the scalar squash chain off the
# critical path between the PE matmul and the next product.
#
# Sharding: pure data parallel, 32 samples per core across 8 cores.
# On-chip layout: partition p = (b4, g) with 4 samples x 32 groups = 128
# partitions; 8 chunks cover the 32 local samples.
#
# Engine placement:
#  - ALL free-axis reductions ride the TensorEngine as PSUM-accumulated
#    matmuls: the g+c contraction for s[b,(j,d)] uses a block-diagonal ones
#    matrix (which also replicates s across the g-partitions for free); the
#    s-sum of the t-step (36 slices), the d-sum of the u-step (16 slices),
#    the c-sum of the db-step (9 slices, absorbing the bias channel), and
#    the j-sum of the softmax normalizer (10 slices) all use an identity
#    matrix. This removes every DVE add-tree.
#  - Every big broadcast-product is split along j between DVE (bf16 2x mode)
#    and GPSIMD/Pool so both engines chew each product concurrently; the
#    split ratios are tuned per product and per iteration (kb/ke/kc/ku/kct).
#  - The routing logits are never materialized: softmax needs only exp(L),
#    so iteration 2 uses et2 = exp(db1) straight from PSUM and iteration 3
#    uses exp(L1+db2) = et2 * exp(db2), removing the logit copy/add from
#    every iteration boundary.
#  - The Activation engine does psum evacuation, exp, square, sqrt.
#
# Schedule: stages are emitted in a wavefront across chunk groups of `bufs`
# (stage-major order) so each engine always has independent work from other
# chunks while one chunk traverses the cross-engine chain.

import sys

import numpy as np

# concourse (Bass/Tile) ships with the container; make sure it's importable
# when the grader runs kernel.py from a bare directory.
for _p in ("/opt/trn_rl_repo", "/root/.axon_site/_ro/trn_rl_repo"):
    if _p not in sys.path:
        sys.path.insert(0, _p)

NS, J, D, C_IN, H, WID, RN = 32, 10, 16, 8, 6, 6, 3
S = H * WID            # 36 spatial positions
CA = C_IN + 1          # 9 channels including the ones channel
CP = 10                # padded channel stride (4B alignment for bf16 rows)
NCORES = 8
BLOC = 32              # samples per core
B4 = 4                 # samples per chunk
NCH = BLOC // B4       # 8 chunks

_CACHE = {}


def _build_program(split_waits=True, kb=(2, 2), ke=(2, 2), kc=(2, 4, 3),
                   ku=(3, 2), kct=(1, 2), et3_pool=False, vt_pool=False,
                   bufs=5, grp=4, grp3=4,
                   dma_eng="sync"):
    kb = (kb, kb) if isinstance(kb, int) else kb
    ke = (ke, ke) if isinstance(ke, int) else ke
    kc = (kc, kc, kc) if isinstance(kc, int) else kc
    ku = (ku, ku) if isinstance(ku, int) else ku
    kct = (kct, kct) if isinstance(kct, int) else kct
    from contextlib import ExitStack

    import concourse.bass as bass
    import concourse.tile as tile
    from concourse import mybir

    f32 = mybir.dt.float32
    bf16 = mybir.dt.float16
    Alu = mybir.AluOpType
    Act = mybir.ActivationFunctionType
    AxX = mybir.AxisListType.X

    nc = bass.Bass("TRN2", target_bir_lowering=True, debug=False,
                   num_devices=NCORES)

    xcs_d = nc.dram_tensor("xcs", [128, NCH * CA * S], bf16,
                           kind="ExternalInput").ap()      # free = (ch, c, s)
    xs1_d = nc.dram_tensor("xs1", [128, NCH * CA], bf16,
                           kind="ExternalInput").ap()      # sum_s x / J
    xsc_d = nc.dram_tensor("xsc", [128, NCH * S * CP], bf16,
                           kind="ExternalInput").ap()      # free = (ch, s, c10)
    wc_d = nc.dram_tensor("wc", [128, J * D * CP], bf16,
                          kind="ExternalInput").ap()       # free = (j, d, c10)
    wu_d = nc.dram_tensor("wu", [128, J * CA * D], bf16,
                          kind="ExternalInput").ap()       # free = (j, c, d)
    ones_d = nc.dram_tensor("onesb", [128, 128], bf16,
                            kind="ExternalInput").ap()     # blockdiag over b4
    eye_d = nc.dram_tensor("eye", [128, 128], bf16,
                           kind="ExternalInput").ap()      # identity
    v_d = nc.dram_tensor("v", [BLOC, J * D], f32,
                         kind="ExternalOutput").ap()

    dmae = {"gpsimd": nc.gpsimd, "sync": nc.sync}[dma_eng]
    with tile.TileContext(nc) as tc, ExitStack() as ctx:
        consts = ctx.enter_context(tc.tile_pool(name="consts", bufs=1))
        xpool = ctx.enter_context(tc.tile_pool(name="xpool", bufs=1))
        spool = ctx.enter_context(tc.tile_pool(name="scratch", bufs=bufs))
        small = ctx.enter_context(tc.tile_pool(name="small", bufs=bufs))
        vpool = ctx.enter_context(tc.tile_pool(name="vpool", bufs=bufs))
        psum = ctx.enter_context(tc.tile_pool(name="psum", bufs=4,
                                              space="PSUM"))

        # The tiny iter-1 seed (xsum/J) first, then the weights the first
        # stages need; the big x tensors are only consumed from iteration 2
        # (pb) / the u-step (pe), so they stream in behind.
        xs1_t = xpool.tile([128, NCH * CA], bf16, tag="xs1t", name="xs1_t")
        nc.gpsimd.dma_start(xs1_t[:, :], xs1_d[:, :])
        wc_t = consts.tile([128, J * D * CP], bf16, tag="wc")
        dmae.dma_start(wc_t[:, :], wc_d[:, :])
        ones_t = consts.tile([128, 128], bf16, tag="onesb")
        dmae.dma_start(ones_t[:, :], ones_d[:, :])
        wu_t = consts.tile([128, J * CA * D], bf16, tag="wu")
        dmae.dma_start(wu_t[:, :], wu_d[:, :])
        eye_t = consts.tile([128, 128], bf16, tag="eye")
        dmae.dma_start(eye_t[:, :], eye_d[:, :])
        xall2 = xpool.tile([128, NCH * S * CP], bf16, tag="xall2",
                           name="xall2")
        nc.gpsimd.dma_start(xall2[:, :], xsc_d[:, :])
        xall = xpool.tile([128, NCH * CA * S], bf16, tag="xall", name="xall")
        nc.gpsimd.dma_start(xall[:, 0:4 * CA * S], xcs_d[:, 0:4 * CA * S])
        dmae.dma_start(xall[:, 4 * CA * S:], xcs_d[:, 4 * CA * S:])

        Xcs = [xall[:, ch * CA * S:(ch + 1) * CA * S] for ch in range(NCH)]
        Xsc = [xall2[:, ch * S * CP:(ch + 1) * S * CP] for ch in range(NCH)]

        # Per-chunk in-flight state handed between stages.
        ST = [dict() for _ in range(NCH)]
        IT = [0]   # current iteration index (0-based), set at emission

        def split_tt(out4, in0, in1, k):
            """Emit a broadcast-product split along the j (outermost free)
            dim: DVE takes j < J-k, Pool takes the last k rows. Both write
            disjoint j-ranges of the same tile."""
            if k < J:
                nc.vector.tensor_tensor(out4[:, 0:J - k], in0[:, 0:J - k],
                                        in1[:, 0:J - k], Alu.mult)
            if k > 0:
                nc.gpsimd.tensor_tensor(out4[:, J - k:J], in0[:, J - k:J],
                                        in1[:, J - k:J], Alu.mult)

        # ---------------- stage functions ----------------

        def st_xs1(ch):
            # iteration-1 seed t1 = sum_s x / J comes in precomputed; this
            # stage only wires up the broadcast view (no instructions).
            ST[ch]["t_b"] = (xs1_t[:, ch * CA:(ch + 1) * CA]
                             .unsqueeze(1).unsqueeze(1)
                             .broadcast_to([128, J, D, CA]))

        def st_pc(ch):
            # prodC[p, (j, d, c)] = t_bcast * Wa
            pc = spool.tile([128, J * D * CP], bf16, tag="prodC", name="pc")
            pc4 = (pc[:, :].rearrange("p (j d c) -> p j d c", j=J, d=D)
                   [:, :, :, 0:CA])
            wc4 = (wc_t[:, :].rearrange("p (j d c) -> p j d c", j=J, d=D)
                   [:, :, :, 0:CA])
            split_tt(pc4, ST[ch]["t_b"], wc4, kc[IT[0]])
            ST[ch]["pc"] = pc

        def st_mm(ch):
            # PE contracts g (partitions, via blockdiag ones) AND c (PSUM
            # accumulation over the 9 channel slices).
            pcz = ST[ch]["pc"][:, :].rearrange("p (a c) -> p a c", c=CP)
            ps = psum.tile([128, J * D], f32, tag="psum_s", name="ps")
            for c in range(CA):
                nc.tensor.matmul(ps[:, :], ones_t[:, :], pcz[:, :, c],
                                 start=(c == 0), stop=(c == CA - 1))
            ST[ch]["ps"] = ps

        def st_scpy(ch):
            # s in bf16 for the u-product, plus s^2 for the squash norm.
            ps = ST[ch]["ps"]
            sb = small.tile([128, J * D], bf16, tag="s_b16", name="sb")
            nc.scalar.copy(sb[:, :], ps[:, :])
            s2 = small.tile([128, J * D], bf16, tag="s2", name="s2")
            nc.scalar.activation(s2[:, :], ps[:, :], Act.Square)
            ST[ch]["sb"] = sb
            ST[ch]["s2"] = s2

        def st_scpy3(ch):
            s2 = small.tile([128, J * D], bf16, tag="s2", name="s2")
            nc.scalar.activation(s2[:, :], ST[ch]["ps"][:, :], Act.Square)
            ST[ch]["s2"] = s2

        def st_n2(ch):
            n2 = small.tile([128, J], f32, tag="n2", name="n2")
            nc.vector.tensor_reduce(
                n2[:, :],
                ST[ch]["s2"][:, :].rearrange("p (j d) -> p j d", j=J),
                AxX, Alu.add)
            ST[ch]["n2"] = n2

        def st_nr(ch):
            nr = small.tile([128, J], f32, tag="nrm", name="nr")
            nc.scalar.activation(nr[:, :], ST[ch]["n2"][:, :], Act.Sqrt)
            ST[ch]["nr"] = nr

        def st_rf(ch):
            n2p1 = small.tile([128, J], f32, tag="n2p1", name="n2p1")
            nc.vector.tensor_scalar_add(n2p1[:, :], ST[ch]["n2"][:, :], 1.0)
            r = small.tile([128, J], f32, tag="rcp", name="r")
            nc.vector.reciprocal(r[:, :], n2p1[:, :])
            f = small.tile([128, J], f32, tag="fac", name="f")
            nc.vector.tensor_tensor(f[:, :], ST[ch]["nr"][:, :], r[:, :],
                                    Alu.mult)
            ST[ch]["f"] = f

        def st_pu(ch):
            # produ[p, (j, c, d)] = s_bcast * Wa  (squash factor applied
            # later, on the d-reduced output). The Pool half reads s from
            # PSUM directly (no SBUF-access penalty on Pool), so it does
            # not wait for the Act bf16 copy.
            k = ku[IT[0]]
            pu = spool.tile([128, J * CA * D], bf16, tag="produ", name="pu")
            pu4 = pu[:, :].rearrange("p (j c d) -> p j c d", j=J, c=CA)
            wu4 = wu_t[:, :].rearrange("p (j c d) -> p j c d", j=J, c=CA)
            sbb = (ST[ch]["sb"][:, :].rearrange("p (j d) -> p j d", j=J)
                   .unsqueeze(2).broadcast_to([128, J, CA, D]))
            split_tt(pu4, sbb, wu4, k)
            ST[ch]["pu"] = pu

        def st_umm(ch):
            # d-reduction (16 slices) on PE via identity-matmul PSUM
            # accumulation, straight off the product.
            puz = ST[ch]["pu"][:, :].rearrange("p (a d) -> p a d", d=D)
            psu = psum.tile([128, J * CA], f32, tag="psum_e", name="psu")
            for k in range(D):
                nc.tensor.matmul(psu[:, :], eye_t[:, :], puz[:, :, k],
                                 start=(k == 0), stop=(k == D - 1))
            ST[ch]["psu"] = psu

        def st_ucp(ch):
            ut = small.tile([128, J * CP], bf16, tag="ut", name="ut")
            ut3 = ut[:, :].rearrange("p (j c) -> p j c", j=J)[:, :, 0:CA]
            nc.scalar.copy(
                ut3, ST[ch]["psu"][:, :].rearrange("p (j c) -> p j c", j=J))
            ST[ch]["ut"] = ut

        def st_uscale(ch):
            # u = f * (s @ W): fold the squash factor into u.
            u = small.tile([128, J * CP], bf16, tag="u", name="u")
            u3 = u[:, :].rearrange("p (j c) -> p j c", j=J)[:, :, 0:CA]
            ut3 = (ST[ch]["ut"][:, :].rearrange("p (j c) -> p j c", j=J)
                   [:, :, 0:CA])
            fb = (ST[ch]["f"][:, :].unsqueeze(2)
                  .broadcast_to([128, J, CA]))
            nc.vector.tensor_tensor(u3, ut3, fb, Alu.mult)
            ST[ch]["u"] = u

        def st_pe(ch):
            # prodE[p, (j, s, c)] over all 9 channels (the ones channel
            # carries u[j,8], summed into db by the PE c-contraction).
            pe = spool.tile([128, J * S * CP], bf16, tag="bigP", name="pe")
            pe4 = (pe[:, :].rearrange("p (j s c) -> p j s c", j=J, s=S)
                   [:, :, :, 0:CA])
            ub = (ST[ch]["u"][:, :].rearrange("p (j c) -> p j c", j=J)
                  [:, :, 0:CA].unsqueeze(2)
                  .broadcast_to([128, J, S, CA]))
            xb = (Xsc[ch].rearrange("p (s c) -> p s c", s=S)
                  [:, :, 0:CA].unsqueeze(1)
                  .broadcast_to([128, J, S, CA]))
            split_tt(pe4, ub, xb, ke[IT[0]])
            ST[ch]["pe"] = pe

        def st_emm(ch):
            # db[p, (j, s)] = sum_c prodE: identity matmul with PSUM
            # accumulation over the 9 channel slices (partition-preserving).
            # In iteration 2 the chain is seeded with the current logits so
            # PSUM accumulates L + db directly (no separate DVE add).
            pez = ST[ch]["pe"][:, :].rearrange("p (a c) -> p a c", c=CP)
            pse = psum.tile([128, J * S], f32, tag="psum_e", name="pse")
            for c in range(CA):
                nc.tensor.matmul(pse[:, :], eye_t[:, :], pez[:, :, c],
                                 start=(c == 0), stop=(c == CA - 1))
            ST[ch]["pse"] = pse

        def st_et2(ch):
            # softmax numerators for iteration 2: exp of the db1 logits,
            # straight from PSUM (the logits are never materialized).
            et = spool.tile([128, J * S], bf16, tag="expt", name="et",
                            bufs=8)
            nc.scalar.activation(et[:, :], ST[ch]["pse"][:, :], Act.Exp)
            ST[ch]["et2"] = et
            ST[ch]["et"] = et

        def st_edb(ch):
            edb = spool.tile([128, J * S], bf16, tag="edb", name="edb")
            nc.scalar.activation(edb[:, :], ST[ch]["pse"][:, :], Act.Exp)
            ST[ch]["edb"] = edb

        def st_et3(ch):
            # exp(L1 + db2) = exp(L1) * exp(db2)
            et = spool.tile([128, J * S], bf16, tag="expt3", name="et3")
            eng = nc.gpsimd if et3_pool else nc.vector
            eng.tensor_tensor(et[:, :], ST[ch]["et2"][:, :],
                              ST[ch]["edb"][:, :], Alu.mult)
            ST[ch]["et"] = et

        def st_zred(ch):
            # z[s] = sum_j exp(L): 10 j-slices accumulated on PE.
            et3 = ST[ch]["et"][:, :].rearrange("p (j s) -> p j s", j=J)
            psz = psum.tile([128, S], f32, tag="psum_s", name="psz")
            for k in range(J):
                nc.tensor.matmul(psz[:, :], eye_t[:, :], et3[:, k, :],
                                 start=(k == 0), stop=(k == J - 1))
            ST[ch]["z"] = psz

        def st_zcp(ch):
            zs = small.tile([128, S], f32, tag="zs", name="zs")
            nc.scalar.copy(zs[:, :], ST[ch]["z"][:, :])
            ST[ch]["zs"] = zs

        def st_zr(ch):
            zr = small.tile([128, S], bf16, tag="zr", name="zr")
            with nc.allow_low_precision("bf16 softmax normalizer"):
                nc.vector.reciprocal(zr[:, :], ST[ch]["zs"][:, :])
            ST[ch]["zr"] = zr

        def st_ct(ch):
            ct = spool.tile([128, J * S], bf16, tag="ct", name="ct")
            zb = ST[ch]["zr"][:, :].unsqueeze(1).broadcast_to([128, J, S])
            split_tt(ct[:, :].rearrange("p (j s) -> p j s", j=J),
                     ST[ch]["et"][:, :].rearrange("p (j s) -> p j s", j=J),
                     zb, kct[IT[0] - 1])
            ST[ch]["ct"] = ct

        def st_pb(ch):
            pb = spool.tile([128, J * CA * S], bf16, tag="bigP", name="pb")
            pb4 = pb[:, :].rearrange("p (j c s) -> p j c s", j=J, c=CA)
            cb = (ST[ch]["ct"][:, :].rearrange("p (j s) -> p j s", j=J)
                  .unsqueeze(2).broadcast_to([128, J, CA, S]))
            xb = (Xcs[ch].rearrange("p (c s) -> p c s", c=CA)
                  .unsqueeze(1).broadcast_to([128, J, CA, S]))
            split_tt(pb4, cb, xb, kb[IT[0] - 1])
            ST[ch]["pb"] = pb

        def st_tmm(ch):
            # s-reduction (all 36 slices) on PE via identity-matmul PSUM
            # accumulation, straight off the product.
            pbz = ST[ch]["pb"][:, :].rearrange("p (a s) -> p a s", s=S)
            pst = psum.tile([128, J * CA], f32, tag="psum_e", name="pst")
            for k in range(S):
                nc.tensor.matmul(pst[:, :], eye_t[:, :], pbz[:, :, k],
                                 start=(k == 0), stop=(k == S - 1))
            ST[ch]["pst"] = pst

        def st_tcp(ch):
            t = small.tile([128, J * CP], bf16, tag="tt", name="t")
            t3 = t[:, :].rearrange("p (j c) -> p j c", j=J)[:, :, 0:CA]
            nc.scalar.copy(
                t3, ST[ch]["pst"][:, :].rearrange("p (j c) -> p j c", j=J))
            ST[ch]["t_b"] = (t[:, :].rearrange("p (j c) -> p j c", j=J)
                             [:, :, 0:CA].unsqueeze(2)
                             .broadcast_to([128, J, D, CA]))

        VOUT = {}

        def st_vt(ch):
            # final v = s * f, fp32, straight from PSUM; chunks of a group
            # write into one tile so the output DMA can be batched.
            g0 = ch - ch % grp3
            if ch == g0:
                VOUT[g0] = vpool.tile([128, grp3 * J * D], f32, tag="vout",
                                      name="vout", bufs=2)
            vt = VOUT[g0][:, (ch - g0) * J * D:(ch - g0 + 1) * J * D]
            fb = (ST[ch]["f"][:, :].unsqueeze(2)
                  .broadcast_to([128, J, D]))
            eng = nc.gpsimd if vt_pool else nc.vector
            eng.tensor_tensor(
                vt.rearrange("p (j d) -> p j d", j=J),
                ST[ch]["ps"][:, :].rearrange("p (j d) -> p j d", j=J), fb,
                Alu.mult)

        def st_out(ch):
            g0 = ch - ch % grp3
            gn = min(grp3, NCH - g0)
            if ch != g0 + gn - 1:
                return
            dst3 = (v_d[g0 * B4:(g0 + gn) * B4, :]
                    .rearrange("(c b) f -> b c f", c=gn, b=B4))
            src3 = (VOUT[g0][0:128:NS, 0:gn * J * D]
                    .rearrange("p (c f) -> p c f", c=gn))
            dmae.dma_start(dst3, src3)

        # ---------------- emission: stage-major wavefront ----------------
        # Chunks are processed in groups of `bufs`: within a group, stages
        # are emitted stage-major (so every engine has independent work from
        # the other chunks of the group), and every tile's consumers are
        # emitted before the next group recycles its buffer.

        def emit(stages, g=None):
            g = grp if g is None else g
            for g0 in range(0, NCH, g):
                for fn in stages:
                    for ch in range(g0, min(g0 + g, NCH)):
                        fn(ch)

        c_sq_u = [st_pc, st_mm, st_scpy, st_n2, st_nr, st_rf, st_pu,
                  st_umm, st_ucp, st_uscale, st_pe, st_emm]
        softmax_b = [st_zred, st_zcp, st_zr, st_ct, st_pb, st_tmm, st_tcp]

        IT[0] = 0
        emit([st_xs1] + c_sq_u)
        IT[0] = 1
        emit([st_et2] + softmax_b + c_sq_u)
        IT[0] = 2
        emit([st_edb, st_et3] + softmax_b +
             [st_pc, st_mm, st_scpy3, st_n2, st_nr, st_rf, st_vt, st_out],
             g=grp3)

    if split_waits:
        _split_multi_waits(nc)
    return nc


def _split_multi_waits(nc):
    """Walrus's cayman codegen allows exactly ONE sync wait per TPB
    instruction (NEURON_ISA_TPB_EVENTS has a single wait slot). Tile's
    scheduler attaches several waits to dependency-merge instructions,
    which the native bass encoder handles but the neuronx-cc path rejects
    ("Too many sync wait commands"). Split the extras onto engine-local
    NoOp instructions inserted immediately before the owner so the wait
    semantics are unchanged.
    """
    from concourse import mybir

    for bbname, bbwrap in nc.bb_map.items():
        bb = bbwrap.bb
        insts = bb.instructions
        i = 0
        while i < len(insts):
            ins = insts[i]
            si = getattr(ins, "sync_info", None)
            if si is None or len(si.on_wait or []) <= 1:
                i += 1
                continue
            waits = list(si.on_wait)
            engine = ins.engine
            for w in waits[:-1]:
                nop = mybir.InstNoOp(
                    name=nc.get_next_instruction_name(),
                    engine=engine,
                    bass_nofuse=True,
                    sync_info=mybir.SyncInfo(on_wait=[w], on_update=[]),
                )
                insts.insert(i, nop)
                i += 1
            ins.sync_info = mybir.SyncInfo(on_wait=[waits[-1]],
                                           on_update=si.on_update)
            i += 1


def _get_program(split_waits=True, **kw):
    key = ("nc", split_waits, tuple(sorted(kw.items())))
    if key not in _CACHE:
        _CACHE[key] = _build_program(split_waits, **kw)
    return _CACHE[key]


def _host_prep(x, W, bias):
    """Build per-core input maps."""
    bf = np.float16
    x = np.ascontiguousarray(x, dtype=np.float32)
    W = np.ascontiguousarray(W, dtype=np.float32)
    bias = np.ascontiguousarray(bias, dtype=np.float32)
    bs = x.shape[0]

    xga = x.reshape(bs, NS, C_IN, S)
    xa = np.concatenate(
        [xga, np.ones((bs, NS, 1, S), dtype=np.float32)], axis=2)
    # [core, ch, b4, g, c, s] -> partition-major [core, b4, g, ch, c, s]
    x6 = xa.reshape(NCORES, NCH, B4, NS, CA, S)
    x6p = x6.transpose(0, 2, 3, 1, 4, 5)
    xcs = np.ascontiguousarray(x6p).reshape(
        NCORES, 128, NCH * CA * S).astype(bf)
    x6sc = x6p.transpose(0, 1, 2, 3, 5, 4)    # [.., ch, s, c]
    x6sp = np.concatenate(
        [x6sc, np.zeros(x6sc.shape[:-1] + (CP - CA,), np.float32)], axis=-1)
    xsc = np.ascontiguousarray(x6sp).reshape(
        NCORES, 128, NCH * S * CP).astype(bf)

    Wa = np.concatenate(
        [W.reshape(NS, J, D, C_IN),
         bias.reshape(NS, J, D, 1)], axis=3)            # [g, j, d, c]
    Wap = np.concatenate(
        [Wa, np.zeros(Wa.shape[:-1] + (CP - CA,), np.float32)], axis=-1)
    wc = np.tile(Wap.reshape(NS, J * D * CP), (B4, 1)).astype(bf)
    wu = np.tile(
        Wa.transpose(0, 1, 3, 2).reshape(NS, J * CA * D),
        (B4, 1)).astype(bf)                             # [128, (j,c,d)]
    onesb = np.kron(np.eye(B4, dtype=np.float32),
                    np.ones((NS, NS), dtype=np.float32)).astype(bf)
    eye = np.eye(128, dtype=np.float32).astype(bf)

    # iteration-1 seed: t1[c] = sum_s x[c,s] / J, per (core, b4, g, ch, c)
    xs1 = np.ascontiguousarray(x6p.sum(axis=5) / J).reshape(
        NCORES, 128, NCH * CA).astype(bf)

    in_maps = [
        {"xcs": np.ascontiguousarray(xcs[k]),
         "xsc": np.ascontiguousarray(xsc[k]),
         "xs1": np.ascontiguousarray(xs1[k]),
         "wc": wc, "wu": wu, "onesb": onesb, "eye": eye}
        for k in range(NCORES)
    ]
    return in_maps


def kernel(x, W, bias, b0):
    from concourse.bass_utils import run_bass_kernel_spmd

    nc = _get_program()
    in_maps = _host_prep(x, W, bias)
    res = run_bass_kernel_spmd(nc, in_maps, list(range(NCORES)))
    out = np.concatenate([res.results[k]["v"] for k in range(NCORES)],
                         axis=0)
    return np.ascontiguousarray(out.reshape(NCORES * BLOC, J, D))
